# revision 1
# baseline (speedup 1.0000x reference)
"""GCN block (GCNConv + LayerNorm + ReLU) on 8 Trainium2 NeuronCores.

Strategy (matches the "shard nodes / partition edges by destination" hint):
  - out = LN(A_norm @ (x @ W^T) + b) = LN((A_norm @ x) @ W^T + b): aggregate
    raw features first (A_norm commutes with the linear map), so the random
    gather runs on node-major x and no transposes are needed anywhere.
  - Destination nodes are sharded contiguously across the 8 cores
    (6250 rows each); each core processes the edges that point into its
    shard.  x is replicated in every core's DRAM as two bf16 gather tables
    (even/odd node rows, so row indices fit dma_gather's int16 indices).
  - Edges are bucketed per 128-destination-node block and padded to whole
    128-edge tiles; multi-block chunks of source rows are fetched with one
    dma_gather per table (output lands tile-major: row j -> partition j%128,
    chunk j//128).  For each 128-edge tile a [128e x 128d] selection matrix
    S (S[e, d] = norm_e if dst_e == d) is built with one fused DVE
    tensor_scalar (iota == dstcol) * norm; the scatter-add is then
    G_cblk^T @ S accumulated in PSUM over the block's tiles, which directly
    yields agg^T laid out as [channel, dst] — exactly the stationary operand
    the W-matmul wants.  agg^T @ W^T gives [dst, out_ch] node-major, and
    bias + LayerNorm + ReLU are fused on DVE/ACT before a contiguous store.
"""

import math
import sys

sys.path.insert(0, "/opt/trn_rl_repo")

import numpy as np
import ml_dtypes

N_NODES = 50000
WIDTH = 256
N_CORES = 8
NODES_PER_CORE = N_NODES // N_CORES  # 6250
P = 128
N_BLOCKS = math.ceil(NODES_PER_CORE / P)  # 49 (last block has 106 rows)
LN_EPS = 1e-5
HALF = N_NODES // 2  # rows per gather table

USE_BF16 = True
GATHER_TILE_CAP = 8  # max tiles (128 idxs each) per dma_gather call (HW limit 1024)


def _preprocess(edge_index):
    """Bucket messages by (core, dst-block, src-parity table), pad each bucket
    to whole 128-edge tiles.

    Processing tile order: per block, even-table tiles then odd-table tiles.
    Gather order: even tiles of all blocks concatenated (ditto odd).
    Returns (TL, TH, dstcol[8,P,Ttot], normv[8,P,Ttot],
             idxe[8,128,8*sum(TL)] i16, idxo[8,128,8*sum(TH)] i16).
    """
    src = np.asarray(edge_index[0]).astype(np.int64)
    dst = np.asarray(edge_index[1]).astype(np.int64)
    loops = np.arange(N_NODES, dtype=np.int64)
    msrc = np.concatenate([src, loops])
    mdst = np.concatenate([dst, loops])

    deg = np.bincount(mdst, minlength=N_NODES).astype(np.float64)
    dinv = 1.0 / np.sqrt(deg)  # deg >= 1 thanks to self loops
    norm = (dinv[msrc] * dinv[mdst]).astype(np.float32)

    core = mdst // NODES_PER_CORE
    r = mdst % NODES_PER_CORE
    blk = np.minimum(r // P, N_BLOCKS - 1)
    dcol = (r - blk * P).astype(np.float32)
    tab = msrc & 1
    gbin = (core * N_BLOCKS + blk) * 2 + tab

    order = np.argsort(gbin, kind="stable")
    msrc, norm, dcol, gbin = msrc[order], norm[order], dcol[order], gbin[order]

    cnt = np.bincount(gbin, minlength=N_CORES * N_BLOCKS * 2).reshape(
        N_CORES, N_BLOCKS, 2
    )
    TL = [int(math.ceil(int(cnt[:, b, 0].max()) / P)) for b in range(N_BLOCKS)]
    TH = [int(math.ceil(int(cnt[:, b, 1].max()) / P)) for b in range(N_BLOCKS)]
    sTL, sTH = sum(TL), sum(TH)
    Ttot = sTL + sTH
    # tile offsets
    EOFF = np.concatenate([[0], np.cumsum(TL)])  # even gather order
    OOFF = np.concatenate([[0], np.cumsum(TH)])  # odd gather order
    TOFF = np.concatenate([[0], np.cumsum(np.asarray(TL) + np.asarray(TH))])

    dstcol = np.zeros((N_CORES, P, Ttot), np.float32)
    normv = np.zeros((N_CORES, P, Ttot), np.float32)
    idxe_flat = np.zeros((N_CORES, sTL * P), np.int16)
    idxo_flat = np.zeros((N_CORES, sTH * P), np.int16)

    starts = np.concatenate([[0], np.cumsum(cnt.ravel())])[:-1]
    j = np.arange(len(gbin)) - starts[gbin]  # index within bucket
    c = gbin // (N_BLOCKS * 2)
    b = (gbin // 2) % N_BLOCKS
    t = gbin & 1
    tile_in_bucket = j // P
    p = j % P
    # metadata in processing order
    tg = np.where(
        t == 0,
        TOFF[b] + tile_in_bucket,
        TOFF[b] + np.asarray(TL)[b] + tile_in_bucket,
    )
    dstcol[c, p, tg] = dcol
    normv[c, p, tg] = norm
    # gather index arrays (per-table tile order)
    idx16 = (msrc >> 1).astype(np.int16)
    Je = (EOFF[b] + tile_in_bucket) * P + p
    Jo = (OOFF[b] + tile_in_bucket) * P + p
    ev = t == 0
    idxe_flat[c[ev], Je[ev]] = idx16[ev]
    idxo_flat[c[~ev], Jo[~ev]] = idx16[~ev]

    # wrap: flat j -> (partition j%16, column j//16), replicated on 8 stripes
    def wrap(flat, ntiles):
        if ntiles == 0:
            return np.zeros((N_CORES, P, 0), np.int16)
        a = flat.reshape(N_CORES, ntiles * 8, 16).transpose(0, 2, 1)  # [8,16,cols]
        return np.ascontiguousarray(np.tile(a, (1, 8, 1)))  # [8,128,cols]

    return TL, TH, dstcol, normv, wrap(idxe_flat, sTL), wrap(idxo_flat, sTH)


def _chunks(TL, TH):
    """Group consecutive blocks into gather chunks where EACH table's tile
    count stays within one dma_gather call's limit."""
    out = []
    cur = []
    ne = no = 0
    for b in range(N_BLOCKS):
        if cur and (ne + TL[b] > GATHER_TILE_CAP or no + TH[b] > GATHER_TILE_CAP):
            out.append((cur, ne, no))
            cur, ne, no = [], 0, 0
        cur.append(b)
        ne += TL[b]
        no += TH[b]
    if cur:
        out.append((cur, ne, no))
    return out


def _build_program(TL, TH, generic_affine, bias_mean):
    import concourse.bass as bass
    import concourse.tile as tile
    from concourse import bacc as bacc_mod
    from concourse import mybir
    from contextlib import ExitStack

    f32 = mybir.dt.float32
    bf16 = mybir.dt.bfloat16
    cdt = bf16 if USE_BF16 else f32
    i16 = mybir.dt.int16
    Alu = mybir.AluOpType
    Act = mybir.ActivationFunctionType
    sTL, sTH = sum(TL), sum(TH)
    Ttot = sTL + sTH
    EOFF = np.concatenate([[0], np.cumsum(TL)])
    OOFF = np.concatenate([[0], np.cumsum(TH)])
    chunks = _chunks(TL, TH)
    max_ne = max(ch[1] for ch in chunks)
    max_no = max(ch[2] for ch in chunks)

    # fcon (f32) column layout: [dst | norm | bias | gamma? | beta?]
    FW = 2 * Ttot + WIDTH + (2 * WIDTH if generic_affine else 0)
    # bcon (cdt) column layout:  [wt_ext 2*(WIDTH+1) | iota (P)]
    BW = 2 * (WIDTH + 1) + P

    nc = bacc_mod.Bacc(None, target_bir_lowering=False, debug=False, num_swdge_queues=4)
    xe_d = nc.declare_dram_parameter("xe", [HALF, WIDTH], cdt, isOutput=False)
    xo_d = nc.declare_dram_parameter("xo", [HALF, WIDTH], cdt, isOutput=False)
    idxe_d = nc.declare_dram_parameter("idxe", [P, 8 * sTL], i16, isOutput=False)
    idxo_d = nc.declare_dram_parameter("idxo", [P, 8 * sTH], i16, isOutput=False)
    fcon_d = nc.declare_dram_parameter("fcon", [P, FW], f32, isOutput=False)
    bcon_d = nc.declare_dram_parameter("bcon", [P, BW], cdt, isOutput=False)
    out_d = nc.declare_dram_parameter("out", [NODES_PER_CORE, WIDTH], f32, isOutput=True)

    with tile.TileContext(nc) as tc:
        with ExitStack() as ctx:
            const = ctx.enter_context(tc.tile_pool(name="const", bufs=1))
            gpool = ctx.enter_context(tc.tile_pool(name="g", bufs=2))
            spool = ctx.enter_context(tc.tile_pool(name="s", bufs=6))
            apool = ctx.enter_context(tc.tile_pool(name="aggT", bufs=2))
            ypool = ctx.enter_context(tc.tile_pool(name="y", bufs=2))
            stat = ctx.enter_context(tc.tile_pool(name="stat", bufs=4))
            ppool = ctx.enter_context(tc.tile_pool(name="psA", bufs=2, space="PSUM"))
            opsum = ctx.enter_context(tc.tile_pool(name="psO", bufs=2, space="PSUM"))

            idxe_sb = const.tile([P, 8 * sTL], i16)
            nc.sync.dma_start(idxe_sb[:], idxe_d[:, :])
            idxo_sb = const.tile([P, 8 * sTH], i16)
            nc.sync.dma_start(idxo_sb[:], idxo_d[:, :])
            fcon_sb = const.tile([P, FW], f32)
            nc.sync.dma_start(fcon_sb[:], fcon_d[:, :])
            bcon_sb = const.tile([P, BW], cdt)
            nc.sync.dma_start(bcon_sb[:], bcon_d[:, :])
            eps_sb = const.tile([P, 1], f32)
            nc.vector.memset(eps_sb[:], LN_EPS)

            bias_sb = fcon_sb[:, 2 * Ttot : 2 * Ttot + WIDTH]
            if generic_affine:
                gamma_sb = fcon_sb[:, 2 * Ttot + WIDTH : 2 * Ttot + 2 * WIDTH]
                beta_sb = fcon_sb[:, 2 * Ttot + 2 * WIDTH : 2 * Ttot + 3 * WIDTH]
            wt_sb = bcon_sb[:, : 2 * (WIDTH + 1)]
            iota_sb = bcon_sb[:, 2 * (WIDTH + 1) : 2 * (WIDTH + 1) + P]
            bmean_sb = const.tile([P, 1], f32)
            nc.vector.memset(bmean_sb[:], bias_mean)

            qn = 0
            for blocks, ne, no in chunks:
                e0 = int(EOFF[blocks[0]])
                o0 = int(OOFF[blocks[0]])
                ge = go = None
                if ne:
                    ge = gpool.tile([P, ne, WIDTH], cdt, tag="ge")
                    nc.gpsimd.dma_gather(
                        ge[:],
                        xe_d[:, :],
                        idxe_sb[:, 8 * e0 : 8 * (e0 + ne)],
                        ne * P,
                        ne * P,
                        WIDTH,
                        queue_num=qn % 4,
                    )
                    qn += 1
                if no:
                    go = gpool.tile([P, no, WIDTH], cdt, tag="go")
                    nc.gpsimd.dma_gather(
                        go[:],
                        xo_d[:, :],
                        idxo_sb[:, 8 * o0 : 8 * (o0 + no)],
                        no * P,
                        no * P,
                        WIDTH,
                        queue_num=qn % 4,
                    )
                    qn += 1
                for b in blocks:
                    tg0 = int(
                        np.concatenate([[0], np.cumsum(np.asarray(TL) + np.asarray(TH))])[
                            b
                        ]
                    )
                    seq = [(ge, int(EOFF[b]) - e0 + t) for t in range(TL[b])] + [
                        (go, int(OOFF[b]) - o0 + t) for t in range(TH[b])
                    ]
                    nt = len(seq)
                    ps0 = ppool.tile([P, P], f32, tag="ps0")
                    ps1 = ppool.tile([P, P], f32, tag="ps1")
                    for k, (gt, col) in enumerate(seq):
                        tg = tg0 + k
                        s = spool.tile([P, P], cdt, tag="s")
                        nc.vector.tensor_scalar(
                            out=s[:],
                            in0=iota_sb,
                            scalar1=fcon_sb[:, tg : tg + 1],
                            scalar2=fcon_sb[:, Ttot + tg : Ttot + tg + 1],
                            op0=Alu.is_equal,
                            op1=Alu.mult,
                        )
                        nc.tensor.matmul(
                            out=ps0[:],
                            lhsT=gt[:, col, 0:P],
                            rhs=s[:],
                            start=(k == 0),
                            stop=(k == nt - 1),
                        )
                        nc.tensor.matmul(
                            out=ps1[:],
                            lhsT=gt[:, col, P:WIDTH],
                            rhs=s[:],
                            start=(k == 0),
                            stop=(k == nt - 1),
                        )
                    # aggT blocks [128 ch, 128 dst] -> SBUF (cast) for W-matmul
                    a0 = apool.tile([P, P], cdt, tag="a0")
                    nc.scalar.copy(a0[:], ps0[:])
                    a1 = apool.tile([P, P], cdt, tag="a1")
                    nc.scalar.copy(a1[:], ps1[:])
                    po = opsum.tile([P, WIDTH + 1], f32, tag="po")
                    nc.tensor.matmul(
                        out=po[:],
                        lhsT=a0[:],
                        rhs=wt_sb[:, : WIDTH + 1],
                        start=True,
                        stop=False,
                    )
                    nc.tensor.matmul(
                        out=po[:],
                        lhsT=a1[:],
                        rhs=wt_sb[:, WIDTH + 1 :],
                        start=False,
                        stop=True,
                    )
                    # ---- epilogue: y = po + bias; LayerNorm; ReLU ----
                    y = ypool.tile([P, WIDTH], f32, tag="y")
                    # NOTE: tensor_tensor_reduce hard-crashes TRN2 here; plain
                    # add, with the row-sum coming free from the W-matmul's
                    # extra weight column (po[:, WIDTH]).
                    nc.vector.tensor_tensor(
                        out=y[:], in0=po[:, :WIDTH], in1=bias_sb, op=Alu.add
                    )
                    sq = ypool.tile([P, WIDTH], f32, tag="sq")
                    ssq = stat.tile([P, 1], f32, tag="ssq")
                    nc.scalar.activation(
                        out=sq[:], in_=y[:], func=Act.Square, accum_out=ssq[:]
                    )
                    mu = stat.tile([P, 1], f32, tag="mu")
                    nc.scalar.activation(
                        out=mu[:],
                        in_=po[:, WIDTH : WIDTH + 1],
                        func=Act.Identity,
                        scale=1.0 / WIDTH,
                        bias=bmean_sb[:, :1],
                    )
                    m2 = stat.tile([P, 1], f32, tag="m2")
                    nc.scalar.square(m2[:], mu[:])
                    var = stat.tile([P, 1], f32, tag="var")
                    nc.vector.tensor_scalar(
                        out=var[:],
                        in0=ssq[:],
                        scalar1=1.0 / WIDTH,
                        scalar2=m2[:, :1],
                        op0=Alu.mult,
                        op1=Alu.subtract,
                    )
                    sd = stat.tile([P, 1], f32, tag="sd")
                    nc.scalar.activation(
                        out=sd[:], in_=var[:], func=Act.Sqrt, bias=eps_sb[:, :1]
                    )
                    rstd = stat.tile([P, 1], f32, tag="rstd")
                    nc.vector.reciprocal(rstd[:], sd[:])
                    t1 = ypool.tile([P, WIDTH], f32, tag="t1")
                    nc.vector.tensor_scalar(
                        out=t1[:],
                        in0=y[:],
                        scalar1=mu[:, :1],
                        scalar2=rstd[:, :1],
                        op0=Alu.subtract,
                        op1=Alu.mult,
                    )
                    if generic_affine:
                        t2 = ypool.tile([P, WIDTH], f32, tag="t2")
                        nc.vector.tensor_tensor(
                            out=t2[:], in0=t1[:], in1=gamma_sb, op=Alu.mult
                        )
                        t3 = ypool.tile([P, WIDTH], f32, tag="t3")
                        nc.vector.tensor_tensor(
                            out=t3[:], in0=t2[:], in1=beta_sb, op=Alu.add
                        )
                        t1 = t3
                    yo = ypool.tile([P, WIDTH], f32, tag="yo")
                    nc.scalar.activation(out=yo[:], in_=t1[:], func=Act.Relu)
                    rows = min(P, NODES_PER_CORE - b * P)
                    nc.sync.dma_start(out_d[b * P : b * P + rows, :], yo[:rows, :])
    return nc


def _pack_inputs(TL, TH, dstcol, normv, idxe, idxo, x, W, bias, gamma, beta, generic_affine):
    cnp = ml_dtypes.bfloat16 if USE_BF16 else np.float32
    Ttot = sum(TL) + sum(TH)

    xc = x.astype(cnp)
    xe = np.ascontiguousarray(xc[0::2])
    xo = np.ascontiguousarray(xc[1::2])
    WT32 = W.T.astype(np.float32)  # [in, out]
    rs = WT32.sum(axis=1, keepdims=True)  # [256, 1] row sums
    WTe = np.concatenate([WT32, rs], axis=1).astype(cnp)  # [256, 257]
    wt = np.concatenate([WTe[:P], WTe[P:]], axis=1)  # [128, 514]
    iota = np.tile(np.arange(P), (P, 1)).astype(cnp)
    bcon = np.ascontiguousarray(np.concatenate([wt, iota], axis=1))

    biasb = np.tile(bias.astype(np.float32)[None, :], (P, 1))
    fparts = [None, None, biasb]
    if generic_affine:
        fparts.append(np.tile(gamma.astype(np.float32)[None, :], (P, 1)))
        fparts.append(np.tile(beta.astype(np.float32)[None, :], (P, 1)))

    in_maps = []
    for c in range(N_CORES):
        fparts[0] = dstcol[c]
        fparts[1] = normv[c]
        fcon = np.ascontiguousarray(np.concatenate(fparts, axis=1, dtype=np.float32))
        in_maps.append(
            {
                "xe": xe,
                "xo": xo,
                "idxe": np.ascontiguousarray(idxe[c]),
                "idxo": np.ascontiguousarray(idxo[c]),
                "fcon": fcon,
                "bcon": bcon,
            }
        )
    return in_maps


_PROGRAM_CACHE = {}


def kernel(x, edge_index, W, b, gamma, beta, _run_kwargs=None):
    from concourse.bass_utils import run_bass_kernel_spmd

    x = np.asarray(x)
    W = np.asarray(W)
    bias = np.asarray(b)
    gamma = np.asarray(gamma)
    beta = np.asarray(beta)

    TL, TH, dstcol, normv, idxe, idxo = _preprocess(edge_index)
    generic_affine = not (np.all(gamma == 1.0) and np.all(beta == 0.0))

    bias_mean = float(bias.astype(np.float64).mean())
    key = (tuple(TL), tuple(TH), generic_affine, bias_mean)
    if key not in _PROGRAM_CACHE:
        nc = _build_program(TL, TH, generic_affine, bias_mean)
        nc.finalize()
        _PROGRAM_CACHE[key] = nc
    nc = _PROGRAM_CACHE[key]

    in_maps = _pack_inputs(
        TL, TH, dstcol, normv, idxe, idxo, x, W, bias, gamma, beta, generic_affine
    )

    kwargs = dict(_run_kwargs or {})
    kwargs.pop("_result", None)
    rr = run_bass_kernel_spmd(nc, in_maps, list(range(N_CORES)), **kwargs)
    out = np.concatenate([rr.results[c]["out"] for c in range(N_CORES)], axis=0)
    if _run_kwargs is not None:
        _run_kwargs["_result"] = rr
    return np.ascontiguousarray(out.astype(np.float32))



# revision 2
# speedup vs baseline: 1.4793x; 1.4793x over previous
"""GCN block (GCNConv + LayerNorm + ReLU) on 8 Trainium2 NeuronCores.

Strategy (v2):
  - out = LN(A_norm @ x @ W^T + b) with A_norm = D^-1/2 A D^-1/2 (self-loops
    included).  LayerNorm is scale-invariant per row, so the dst-side scaling
    dinv[dst] can be dropped entirely if the bias is pre-scaled per row:
    LN(dinv_d * (A_d + sqrt(deg_d) * b)) == LN(A_d + sqrt(deg_d) * b), where
    A_d = sum_{e->d} dinv[src] x[src] + dinv[d] x[d].
  - dinv[src] is folded into the gather table on the host (xs = dinv * x,
    bf16), which makes every scatter matrix PURELY BINARY -> precomputed on
    the host and streamed to SBUF as fp8 (0/1 exact), eliminating the DVE
    tensor_scalar builds that dominated v1.
  - Destination nodes sharded contiguously across 8 cores (6250 rows each,
    49 blocks of 128).  Edges bucketed per (dst-block, src-parity table) and
    padded to 128-edge tiles; source rows fetched with dma_gather (bf16,
    even/odd tables so row indices fit int16).
  - Per 128-edge tile: aggT += G^T @ S accumulated in PSUM ([ch, dst]); the
    self-loop contribution comes from a contiguous shard load used as lhsT
    against streamed identity tiles (keeps 6250 rows/core out of the random
    gather).  aggT @ W^T plus a rank-1 bias matmul (sqrtdeg x [b|sum b])
    yields po = A + sqrt(deg) b with a free row-sum column for the LN mean.
  - Epilogue: Square+accum (ACT) for ssq, small DVE ops for mu/var/rstd,
    one fused ACT Relu(po * rstd + (-mu * rstd)) pass, contiguous store.
  - Emission is software-pipelined: block b's aggregation matmuls are
    emitted before block b-1's W-matmul so the tensor engine never waits
    on the PSUM->SBUF copies.
"""

import math
import sys

sys.path.insert(0, "/opt/trn_rl_repo")

import numpy as np
import ml_dtypes

N_NODES = 50000
WIDTH = 256
N_CORES = 8
NODES_PER_CORE = N_NODES // N_CORES  # 6250
P = 128
N_BLOCKS = math.ceil(NODES_PER_CORE / P)  # 49 (last block has 106 rows)
LN_EPS = 1e-5
HALF = N_NODES // 2  # rows per gather table

GATHER_TILE_CAP = 8  # max tiles (128 idxs each) per dma_gather call
S_SPLIT = (0, 4, 16, N_BLOCKS)  # S stream split points (block ranges)

_f8np = ml_dtypes.float8_e4m3
_bfnp = ml_dtypes.bfloat16


def _preprocess(edge_index):
    """Bucket non-self-loop edges by (core, dst-block, src-parity table), pad
    to whole 128-edge tiles, and build the binary scatter-tile stream.

    Per-block processing tile order: even-table tiles, odd-table tiles, then
    one self-loop identity tile (lhsT = contiguous shard rows).
    """
    src = np.asarray(edge_index[0]).astype(np.int64)
    dst = np.asarray(edge_index[1]).astype(np.int64)

    deg = np.bincount(dst, minlength=N_NODES).astype(np.float64) + 1.0  # + self
    dinv = 1.0 / np.sqrt(deg)
    sqdeg_all = np.sqrt(deg)

    core = dst // NODES_PER_CORE
    r = dst % NODES_PER_CORE
    blk = np.minimum(r // P, N_BLOCKS - 1)
    dcol = r - blk * P
    tab = src & 1
    gbin = (core * N_BLOCKS + blk) * 2 + tab

    order = np.argsort(gbin, kind="stable")
    src_s, dcol_s, gbin_s = src[order], dcol[order], gbin[order]

    cnt = np.bincount(gbin, minlength=N_CORES * N_BLOCKS * 2).reshape(
        N_CORES, N_BLOCKS, 2
    )
    TL = [int(math.ceil(int(cnt[:, b, 0].max()) / P)) for b in range(N_BLOCKS)]
    TH = [int(math.ceil(int(cnt[:, b, 1].max()) / P)) for b in range(N_BLOCKS)]
    sTL, sTH = sum(TL), sum(TH)
    stot = sTL + sTH + N_BLOCKS
    EOFF = np.concatenate([[0], np.cumsum(TL)])  # even gather tile offsets
    OOFF = np.concatenate([[0], np.cumsum(TH)])  # odd gather tile offsets
    TOFF = np.concatenate(
        [[0], np.cumsum(np.asarray(TL) + np.asarray(TH) + 1)]
    )  # S-stream tile offsets (incl self tiles)

    starts = np.concatenate([[0], np.cumsum(cnt.ravel())])[:-1]
    j = np.arange(len(gbin_s)) - starts[gbin_s]  # index within bucket
    c = gbin_s // (N_BLOCKS * 2)
    b = (gbin_s // 2) % N_BLOCKS
    t = gbin_s & 1
    til = j // P
    p = j % P

    # gather index arrays (per-table tile order)
    idxe_flat = np.zeros((N_CORES, sTL * P), np.int16)
    idxo_flat = np.zeros((N_CORES, sTH * P), np.int16)
    idx16 = (src_s >> 1).astype(np.int16)
    Je = (EOFF[b] + til) * P + p
    Jo = (OOFF[b] + til) * P + p
    ev = t == 0
    idxe_flat[c[ev], Je[ev]] = idx16[ev]
    idxo_flat[c[~ev], Jo[~ev]] = idx16[~ev]

    # binary S stream: [core, P, stot*P] fp8
    S_all = np.zeros((N_CORES, P, stot * P), _f8np)
    tg = np.where(t == 0, TOFF[b] + til, TOFF[b] + np.asarray(TL)[b] + til)
    S_all[c, p, tg * P + dcol_s] = _f8np(1.0)
    # self-loop identity tiles (same pattern for every core)
    for bb in range(N_BLOCKS):
        rows = min(P, NODES_PER_CORE - bb * P)
        ts = TOFF[bb] + TL[bb] + TH[bb]
        pr = np.arange(rows)
        S_all[:, pr, ts * P + pr] = _f8np(1.0)

    # wrap: flat j -> (partition j%16, column j//16), replicated on 8 stripes
    def wrap(flat, ntiles):
        if ntiles == 0:
            return np.zeros((N_CORES, P, 0), np.int16)
        a = flat.reshape(N_CORES, ntiles * 8, 16).transpose(0, 2, 1)
        return np.ascontiguousarray(np.tile(a, (1, 8, 1)))  # [8,128,cols]

    return (
        TL,
        TH,
        dinv,
        sqdeg_all,
        S_all,
        wrap(idxe_flat, sTL),
        wrap(idxo_flat, sTH),
    )


def _chunks(TL, TH):
    """Group consecutive blocks into gather chunks where EACH table's tile
    count stays within one dma_gather call's limit."""
    out = []
    cur = []
    ne = no = 0
    for b in range(N_BLOCKS):
        if cur and (ne + TL[b] > GATHER_TILE_CAP or no + TH[b] > GATHER_TILE_CAP):
            out.append((cur, ne, no))
            cur, ne, no = [], 0, 0
        cur.append(b)
        ne += TL[b]
        no += TH[b]
    if cur:
        out.append((cur, ne, no))
    return out


def _build_program(TL, TH, generic_affine):
    import concourse.bass as bass
    import concourse.tile as tile
    from concourse import bacc as bacc_mod
    from concourse import mybir
    from contextlib import ExitStack

    f32 = mybir.dt.float32
    bf16 = mybir.dt.bfloat16
    f8 = mybir.dt.float8e4
    i16 = mybir.dt.int16
    Alu = mybir.AluOpType
    Act = mybir.ActivationFunctionType

    sTL, sTH = sum(TL), sum(TH)
    stot = sTL + sTH + N_BLOCKS
    EOFF = np.concatenate([[0], np.cumsum(TL)])
    OOFF = np.concatenate([[0], np.cumsum(TH)])
    TOFF = np.concatenate([[0], np.cumsum(np.asarray(TL) + np.asarray(TH) + 1)])
    chunks = _chunks(TL, TH)

    # S stream split into 3 dram params so early blocks' tiles arrive fast
    s_sizes = [int(TOFF[S_SPLIT[i + 1]] - TOFF[S_SPLIT[i]]) * P for i in range(3)]

    nc = bacc_mod.Bacc(None, target_bir_lowering=False, debug=False, num_swdge_queues=4)
    xe_d = nc.declare_dram_parameter("xe", [HALF, WIDTH], bf16, isOutput=False)
    xo_d = nc.declare_dram_parameter("xo", [HALF, WIDTH], bf16, isOutput=False)
    idxe_d = nc.declare_dram_parameter("idxe", [P, 8 * sTL], i16, isOutput=False)
    idxo_d = nc.declare_dram_parameter("idxo", [P, 8 * sTH], i16, isOutput=False)
    s_d = [
        nc.declare_dram_parameter(f"s{i}", [P, s_sizes[i]], f8, isOutput=False)
        for i in range(3)
    ]
    xsh_d = nc.declare_dram_parameter("xsh", [P, N_BLOCKS * WIDTH], bf16, isOutput=False)
    wt_d = nc.declare_dram_parameter("wt", [P, 2 * (WIDTH + 1)], bf16, isOutput=False)
    brow_d = nc.declare_dram_parameter("brow", [1, WIDTH + 1], bf16, isOutput=False)
    sqd_d = nc.declare_dram_parameter("sqdeg", [1, N_BLOCKS * P], bf16, isOutput=False)
    if generic_affine:
        gb_d = nc.declare_dram_parameter("gb", [P, 2 * WIDTH], f32, isOutput=False)
    out_d = nc.declare_dram_parameter("out", [NODES_PER_CORE, WIDTH], f32, isOutput=True)

    def s_tile_ap(s_sb, tg):
        """SBUF AP of S-stream tile number tg."""
        for i in range(3):
            lo, hi = int(TOFF[S_SPLIT[i]]), int(TOFF[S_SPLIT[i + 1]])
            if lo <= tg < hi:
                off = (tg - lo) * P
                return s_sb[i][:, off : off + P]
        raise AssertionError(tg)

    with tile.TileContext(nc) as tc:
        with ExitStack() as ctx:
            const = ctx.enter_context(tc.tile_pool(name="const", bufs=1))
            gpool = ctx.enter_context(tc.tile_pool(name="g", bufs=3))
            apool = ctx.enter_context(tc.tile_pool(name="aggT", bufs=3))
            ypool = ctx.enter_context(tc.tile_pool(name="y", bufs=3))
            stat = ctx.enter_context(tc.tile_pool(name="stat", bufs=4))
            ppool = ctx.enter_context(tc.tile_pool(name="psA", bufs=2, space="PSUM"))
            opsum = ctx.enter_context(tc.tile_pool(name="psO", bufs=2, space="PSUM"))

            idxe_sb = const.tile([P, 8 * sTL], i16)
            nc.sync.dma_start(idxe_sb[:], idxe_d[:, :])
            idxo_sb = const.tile([P, 8 * sTH], i16)
            nc.sync.dma_start(idxo_sb[:], idxo_d[:, :])
            s_sb = []
            for i in range(3):
                t = const.tile([P, s_sizes[i]], f8)
                nc.sync.dma_start(t[:], s_d[i][:, :])
                s_sb.append(t)
            wt_sb = const.tile([P, 2 * (WIDTH + 1)], bf16)
            nc.sync.dma_start(wt_sb[:], wt_d[:, :])
            brow_sb = const.tile([1, WIDTH + 1], bf16)
            nc.sync.dma_start(brow_sb[:], brow_d[:, :])
            sqd_sb = const.tile([1, N_BLOCKS * P], bf16)
            nc.sync.dma_start(sqd_sb[:], sqd_d[:, :])
            xsh_sb = const.tile([P, N_BLOCKS * WIDTH], bf16)
            nc.sync.dma_start(xsh_sb[:], xsh_d[:, :])
            if generic_affine:
                gb_sb = const.tile([P, 2 * WIDTH], f32)
                nc.sync.dma_start(gb_sb[:], gb_d[:, :])
                gamma_sb = gb_sb[:, :WIDTH]
                beta_sb = gb_sb[:, WIDTH:]
            eps_sb = const.tile([P, 1], f32)
            nc.vector.memset(eps_sb[:], LN_EPS)

            def emit_tail(b, a0, a1):
                """W-matmul + rank-1 bias + LN/ReLU epilogue + store for b."""
                po = opsum.tile([P, WIDTH + 1], f32, tag="po")
                nc.tensor.matmul(
                    out=po[:], lhsT=a0[:], rhs=wt_sb[:, : WIDTH + 1],
                    start=True, stop=False,
                )
                nc.tensor.matmul(
                    out=po[:], lhsT=a1[:], rhs=wt_sb[:, WIDTH + 1 :],
                    start=False, stop=False,
                )
                nc.tensor.matmul(
                    out=po[:],
                    lhsT=sqd_sb[0:1, b * P : (b + 1) * P],
                    rhs=brow_sb[0:1, :],
                    start=False, stop=True,
                )
                # ---- LN epilogue: po rows are A + sqrt(deg) b ----
                sq = ypool.tile([P, WIDTH], f32, tag="sq")
                ssq = stat.tile([P, 1], f32, tag="ssq")
                nc.scalar.activation(
                    out=sq[:], in_=po[:, :WIDTH], func=Act.Square, accum_out=ssq[:]
                )
                mu = stat.tile([P, 1], f32, tag="mu")
                nc.vector.tensor_scalar(
                    out=mu[:], in0=po[:, WIDTH : WIDTH + 1],
                    scalar1=1.0 / WIDTH, scalar2=None, op0=Alu.mult,
                )
                m2 = stat.tile([P, 1], f32, tag="m2")
                nc.vector.tensor_scalar(
                    out=m2[:], in0=mu[:], scalar1=mu[:, :1], scalar2=None,
                    op0=Alu.mult,
                )
                var = stat.tile([P, 1], f32, tag="var")
                nc.vector.tensor_scalar(
                    out=var[:], in0=ssq[:], scalar1=1.0 / WIDTH,
                    scalar2=m2[:, :1], op0=Alu.mult, op1=Alu.subtract,
                )
                sd = stat.tile([P, 1], f32, tag="sd")
                nc.scalar.activation(
                    out=sd[:], in_=var[:], func=Act.Sqrt, bias=eps_sb[:, :1]
                )
                rstd = stat.tile([P, 1], f32, tag="rstd")
                nc.vector.reciprocal(rstd[:], sd[:])
                mrs = stat.tile([P, 1], f32, tag="mrs")
                nc.vector.tensor_scalar(
                    out=mrs[:], in0=mu[:], scalar1=rstd[:, :1], scalar2=-1.0,
                    op0=Alu.mult, op1=Alu.mult,
                )
                yo = ypool.tile([P, WIDTH], f32, tag="yo")
                if generic_affine:
                    t1 = ypool.tile([P, WIDTH], f32, tag="t1")
                    nc.scalar.activation(
                        out=t1[:], in_=po[:, :WIDTH], func=Act.Identity,
                        scale=rstd[:, :1], bias=mrs[:, :1],
                    )
                    t2 = ypool.tile([P, WIDTH], f32, tag="t2")
                    nc.vector.tensor_tensor(
                        out=t2[:], in0=t1[:], in1=gamma_sb, op=Alu.mult
                    )
                    t3 = ypool.tile([P, WIDTH], f32, tag="t3")
                    nc.vector.tensor_tensor(
                        out=t3[:], in0=t2[:], in1=beta_sb, op=Alu.add
                    )
                    nc.scalar.activation(out=yo[:], in_=t3[:], func=Act.Relu)
                else:
                    nc.scalar.activation(
                        out=yo[:], in_=po[:, :WIDTH], func=Act.Relu,
                        scale=rstd[:, :1], bias=mrs[:, :1],
                    )
                rows = min(P, NODES_PER_CORE - b * P)
                nc.sync.dma_start(out_d[b * P : b * P + rows, :], yo[:rows, :])

            qn = 0
            pending = None  # (b, a0, a1) awaiting W-matmul + epilogue
            for blocks, ne, no in chunks:
                e0 = int(EOFF[blocks[0]])
                o0 = int(OOFF[blocks[0]])
                ge = go = None
                if ne:
                    ge = gpool.tile([P, ne, WIDTH], bf16, tag="ge")
                    nc.gpsimd.dma_gather(
                        ge[:], xe_d[:, :],
                        idxe_sb[:, 8 * e0 : 8 * (e0 + ne)],
                        ne * P, ne * P, WIDTH, queue_num=qn % 4,
                    )
                    qn += 1
                if no:
                    go = gpool.tile([P, no, WIDTH], bf16, tag="go")
                    nc.gpsimd.dma_gather(
                        go[:], xo_d[:, :],
                        idxo_sb[:, 8 * o0 : 8 * (o0 + no)],
                        no * P, no * P, WIDTH, queue_num=qn % 4,
                    )
                    qn += 1
                for b in blocks:
                    tg0 = int(TOFF[b])
                    seq = (
                        [(ge, int(EOFF[b]) - e0 + t, tg0 + t) for t in range(TL[b])]
                        + [
                            (go, int(OOFF[b]) - o0 + t, tg0 + TL[b] + t)
                            for t in range(TH[b])
                        ]
                    )
                    nt = len(seq) + 1  # + self tile
                    ps0 = ppool.tile([P, P], f32, tag="ps0")
                    ps1 = ppool.tile([P, P], f32, tag="ps1")
                    for k, (gt, col, tg) in enumerate(seq):
                        s_ap = s_tile_ap(s_sb, tg)
                        nc.tensor.matmul(
                            out=ps0[:], lhsT=gt[:, col, 0:P], rhs=s_ap,
                            start=(k == 0), stop=False,
                        )
                        nc.tensor.matmul(
                            out=ps1[:], lhsT=gt[:, col, P:WIDTH], rhs=s_ap,
                            start=(k == 0), stop=False,
                        )
                    # self-loop tile: lhsT = contiguous shard rows
                    tg_self = tg0 + TL[b] + TH[b]
                    s_ap = s_tile_ap(s_sb, tg_self)
                    nc.tensor.matmul(
                        out=ps0[:],
                        lhsT=xsh_sb[:, b * WIDTH : b * WIDTH + P],
                        rhs=s_ap, start=(nt == 1), stop=True,
                    )
                    nc.tensor.matmul(
                        out=ps1[:],
                        lhsT=xsh_sb[:, b * WIDTH + P : (b + 1) * WIDTH],
                        rhs=s_ap, start=(nt == 1), stop=True,
                    )
                    # aggT -> SBUF (cast to bf16) for the W-matmul
                    a0 = apool.tile([P, P], bf16, tag="a0")
                    nc.scalar.copy(a0[:], ps0[:])
                    a1 = apool.tile([P, P], bf16, tag="a1")
                    nc.vector.tensor_copy(a1[:], ps1[:])
                    if pending is not None:
                        emit_tail(*pending)
                    pending = (b, a0, a1)
            emit_tail(*pending)
    return nc


def _pack_inputs(TL, TH, dinv, sqdeg_all, S_all, idxe, idxo, x, W, bias, gamma, beta, generic_affine):
    sTL, sTH = sum(TL), sum(TH)
    stot = sTL + sTH + N_BLOCKS
    TOFF = np.concatenate([[0], np.cumsum(np.asarray(TL) + np.asarray(TH) + 1)])

    xs = (dinv[:, None] * x.astype(np.float64)).astype(_bfnp)
    xe = np.ascontiguousarray(xs[0::2])
    xo = np.ascontiguousarray(xs[1::2])

    WT32 = W.T.astype(np.float32)  # [in, out]
    rs = WT32.sum(axis=1, keepdims=True)  # [256, 1] row sums
    WTe = np.concatenate([WT32, rs], axis=1).astype(_bfnp)  # [256, 257]
    wt = np.ascontiguousarray(np.concatenate([WTe[:P], WTe[P:]], axis=1))
    b32 = bias.astype(np.float32)
    brow = np.concatenate([b32, [b32.sum()]])[None, :].astype(_bfnp)

    s_arrs = [
        np.ascontiguousarray(
            S_all[:, :, int(TOFF[S_SPLIT[i]]) * P : int(TOFF[S_SPLIT[i + 1]]) * P]
        )
        for i in range(3)
    ]

    if generic_affine:
        gb = np.concatenate(
            [
                np.tile(gamma.astype(np.float32)[None, :], (P, 1)),
                np.tile(beta.astype(np.float32)[None, :], (P, 1)),
            ],
            axis=1,
        )

    in_maps = []
    for c in range(N_CORES):
        lo = c * NODES_PER_CORE
        # contiguous shard rows, [128, 49*256]: xsh[p, b*256+ch] = xs[lo+b*128+p, ch]
        xsh = np.zeros((N_BLOCKS * P, WIDTH), _bfnp)
        xsh[:NODES_PER_CORE] = xs[lo : lo + NODES_PER_CORE]
        xsh = np.ascontiguousarray(
            xsh.reshape(N_BLOCKS, P, WIDTH).transpose(1, 0, 2).reshape(P, -1)
        )
        sq = np.zeros((1, N_BLOCKS * P), _bfnp)
        sq[0, :NODES_PER_CORE] = sqdeg_all[lo : lo + NODES_PER_CORE].astype(_bfnp)
        sq[0, NODES_PER_CORE:] = _bfnp(1.0)
        m = {
            "xe": xe,
            "xo": xo,
            "idxe": np.ascontiguousarray(idxe[c]),
            "idxo": np.ascontiguousarray(idxo[c]),
            "s0": s_arrs[0][c],
            "s1": s_arrs[1][c],
            "s2": s_arrs[2][c],
            "xsh": xsh,
            "wt": wt,
            "brow": brow,
            "sqdeg": sq,
        }
        if generic_affine:
            m["gb"] = gb
        in_maps.append(m)
    return in_maps


_PROGRAM_CACHE = {}


def kernel(x, edge_index, W, b, gamma, beta, _run_kwargs=None):
    from concourse.bass_utils import run_bass_kernel_spmd

    x = np.asarray(x)
    W = np.asarray(W)
    bias = np.asarray(b)
    gamma = np.asarray(gamma)
    beta = np.asarray(beta)

    TL, TH, dinv, sqdeg_all, S_all, idxe, idxo = _preprocess(edge_index)
    generic_affine = not (np.all(gamma == 1.0) and np.all(beta == 0.0))

    key = (tuple(TL), tuple(TH), generic_affine)
    if key not in _PROGRAM_CACHE:
        nc = _build_program(TL, TH, generic_affine)
        nc.finalize()
        _PROGRAM_CACHE[key] = nc
    nc = _PROGRAM_CACHE[key]

    in_maps = _pack_inputs(
        TL, TH, dinv, sqdeg_all, S_all, idxe, idxo, x, W, bias, gamma, beta,
        generic_affine,
    )

    kwargs = dict(_run_kwargs or {})
    kwargs.pop("_result", None)
    rr = run_bass_kernel_spmd(nc, in_maps, list(range(N_CORES)), **kwargs)
    out = np.concatenate([rr.results[c]["out"] for c in range(N_CORES)], axis=0)
    if _run_kwargs is not None:
        _run_kwargs["_result"] = rr
    return np.ascontiguousarray(out.astype(np.float32))
